# revision 6
# baseline (speedup 1.0000x reference)
"""Trainium2 Bass kernel for nn_LongTermFeatureBank (8-core SPMD, batch-sharded).

v2: host-packed layouts (one fat descriptor per partition for every DMA),
bf16 attention/classifier matmuls with f32 PSUM accumulate, flat x loads
([b,d2,d1,hw] pre-transposed on host) + TensorE transposes, collectives
hidden behind independent matmul work.
"""
import os
import numpy as np
import ml_dtypes

import concourse.bass as bass
import concourse.tile as tile
from concourse import bacc, mybir
from concourse.bass_utils import run_bass_kernel_spmd

F32 = mybir.dt.float32
BF16 = mybir.dt.bfloat16
X = mybir.AxisListType.X
ADD = mybir.AluOpType.add
SUB = mybir.AluOpType.subtract
MUL = mybir.AluOpType.mult
BYP = mybir.AluOpType.bypass
EXP = mybir.ActivationFunctionType.Exp
RELU = mybir.ActivationFunctionType.Relu
SQRT = mybir.ActivationFunctionType.Sqrt
SQUARE = mybir.ActivationFunctionType.Square
CPY = mybir.ActivationFunctionType.Copy

B, D1, D2, HW = 128, 512, 16, 49
HWP = 50              # 49 padded to 50 (dup elem 0) -> even groups, DVE 2x mode
NC = 8
BL = B // NC          # 16 local batch
R = BL * D2           # 256 rows (b,d2)
ISD = float(1.0 / np.sqrt(D1))
CHK = D1 // 4 * HWP   # 6400 bf16 per partition per x sub-chunk
DSPL = 80             # d-split within a chunk: vector does 0:80, gpsimd 80:128
LAST = {}


def _declare(nc):
    ap = {}
    def di(name, shape, dt=F32):
        ap[name] = nc.dram_tensor(name, list(shape), dt, kind="ExternalInput").ap()
    di("x_s", (BL, D2, D1 * HWP), BF16)
    for s in range(2):
        di(f"wa{s}", (128, 4, D1), BF16); di(f"wb{s}", (128, 4, D1), BF16)
        di(f"wf{s}", (128, 4, D1), BF16)
        di(f"bar{s}", (1, D1), BF16); di(f"bbr{s}", (1, D1), BF16)
        di(f"bfr{s}", (1, D1), BF16)
    di("lngT", (128, 4, D2)); di("lnbT", (128, 4, D2))
    di("w1a", (128, 64, 200), BF16); di("w1b", (128, 64, 200), BF16)
    di("b1", (1, 200), BF16)
    di("w2T", (200, 50), BF16); di("b2T", (50, 1))
    di("w3T", (50, 10), BF16); di("b3T", (10, 1))
    di("w4T", (10, 128), BF16); di("b4T", (128, 1))
    di("p8", (128, 16), BF16); di("p8t", (16, 128), BF16)
    di("maskbd", (128, 128), BF16)
    di("idn", (128, 128), BF16); di("ones", (1, 256), BF16); di("ones128", (128, 1))
    di("onesr", (1, 128))
    ap["outT"] = nc.dram_tensor("outT", [128, BL], F32, kind="ExternalOutput").ap()
    return ap


def _build(nc, tc, ap, collective=True):
    MM = nc.tensor.matmul
    cp = nc.vector.tensor_copy
    tt = nc.vector.tensor_tensor
    gt = nc.gpsimd.tensor_tensor
    act = nc.scalar.activation
    from contextlib import ExitStack
    ctx = ExitStack()
    P = ctx.enter_context(tc.tile_pool(name="persist", bufs=1))
    PX = ctx.enter_context(tc.tile_pool(name="xstage", bufs=6))
    PP = ctx.enter_context(tc.tile_pool(name="ps", bufs=6, space="PSUM"))
    PH = ctx.enter_context(tc.tile_pool(name="ph", bufs=1, space="PSUM"))
    PD = ctx.enter_context(tc.tile_pool(name="dram", bufs=4, space="DRAM"))

    def ps(tag="ps"):
        return PP.tile([128, 512], F32, tag=tag, name=tag)

    # ---- persistent SBUF loads: weights on the Act HWDGE ring ----
    def ld(name, shape, src=None, dt=None, eng=None):
        src = src if src is not None else ap[name]
        t = P.tile(list(shape), dt or src.dtype, tag=name)
        (eng or nc.scalar).dma_start(t[:], src)
        return t
    w = {}
    # earliest-needed first (x data shares the fabric; these are small)
    for s in range(2):
        w[f"wa{s}"] = ld(f"wa{s}", (128, 4, D1))
        w[f"wb{s}"] = ld(f"wb{s}", (128, 4, D1))
        w[f"bar{s}"] = ld(f"bar{s}", (1, D1))
        w[f"bbr{s}"] = ld(f"bbr{s}", (1, D1))
        w[f"wf{s}"] = ld(f"wf{s}", (128, 4, D1))
        w[f"bfr{s}"] = ld(f"bfr{s}", (1, D1))
    ones = ld("ones", (1, 256))
    p8 = ld("p8", (128, 16))
    p8t = ld("p8t", (16, 128))
    maskbd = ld("maskbd", (128, 128))
    idn = ld("idn", (128, 128))
    lngT = ld("lngT", (128, 4, D2))
    lnbT = ld("lnbT", (128, 4, D2))
    ones128 = ld("ones128", (128, 1))
    onesr = ld("onesr", (1, 128))
    b1 = ld("b1", (1, 200))
    w2a = ld("w2a", (128, 50), ap["w2T"][0:128, :])
    w2b = ld("w2b", (72, 50), ap["w2T"][128:200, :])
    w3 = ld("w3", (50, 10), ap["w3T"]); w4 = ld("w4", (10, 128), ap["w4T"])
    b2T = ld("b2T", (50, 1), ap["b2T"]); b3T = ld("b3T", (10, 1), ap["b3T"])
    b4T = ld("b4T", (128, 1), ap["b4T"])

    # ---- x: 8 flat sub-chunk DMAs on the SP ring; pool on vector ----
    # host layout x_s = [b, d2, d1*hw]; sub-chunk (g, j): 128 partitions
    # (b,d2) in half g, d1 quarter j — contiguous 25 KB per partition.
    # warm-up collective: absorbs CC cold-start + aligns cores during DMA
    if collective:
        warm = P.tile([16, 16], F32, tag="warm", name="warm")
        nc.vector.memset(warm[:], 1.0)
        ibw = PD.tile([16, 16], F32, tag="agwin", name="agwin")
        obw = PD.tile([128, 16], F32, tag="agwout", name="agwout")
        nc.sync.dma_start(ibw[:], warm[:])
        nc.gpsimd.collective_compute(
            "AllGather", BYP, ins=[ibw.opt()], outs=[obw.opt()],
            replica_groups=[list(range(NC))])

    xr = ap["x_s"].rearrange("b d f -> (b d) f")
    xps = [P.tile([128, D1], BF16, tag=f"xps{g}", name=f"xps{g}") for g in range(2)]
    for g in range(2):
        for j in range(4):
            xt = PX.tile([128, CHK], BF16, tag="xt", name="xt")
            nc.sync.dma_start(xt[:], xr[g * 128:(g + 1) * 128,
                                         j * CHK:(j + 1) * CHK])
            nc.vector.reduce_max(
                out=xps[g][:, j * 128:(j + 1) * 128],
                in_=xt.rearrange("p (d h) -> p d h", h=HWP), axis=X)

    # w1 after x on the SP ring: x gets full bandwidth first
    w1a = ld("w1a", (128, 64, 200), eng=nc.sync)
    w1b = ld("w1b", (128, 64, 200), eng=nc.sync)

    # xpT[c] [128 d1, 256 (b,d2)] bf16 — written per-half by head0 below
    xpT = [P.tile([128, R], BF16, tag=f"xpT{c}", name=f"xpT{c}") for c in range(4)]

    # ---- classifier first-layer accumulate (interleaved for AG hiding) ----
    ph1 = PH.tile([128, 512], F32, tag="ph1", name="ph1")
    w1t = {0: w1a, 1: w1b}
    def cls_mm(j0, j1, fxs):
        for j in range(j0, j1):
            which, d2, c = j >> 6, (j >> 2) & 15, j & 3
            grp = j % 4
            lhsT = fxs[which][c].rearrange("p (b c2) -> p c2 b", c2=16)[:, d2, :]
            MM(ph1[32 * grp:32 * grp + 16, :200], lhsT,
               w1t[which][:, j & 63, :],
               start=(j < 4), stop=(j >= 125), tile_position=(0, 32 * grp),
               skip_group_check=True)

    def proj(wm, brow, src, tag):
        out = []
        for oc in range(4):
            pa = ps()
            for k in range(4):
                MM(pa[:, :R], wm[:, k, oc * 128:(oc + 1) * 128], src[k][:],
                   start=(k == 0), stop=False)
            MM(pa[:, :R], brow[0:1, oc * 128:(oc + 1) * 128], ones[0:1, :R],
               start=False, stop=True)
            t = P.tile([128, R], BF16, tag=f"{tag}{oc}", name=f"{tag}{oc}")
            act(out=t[:], in_=pa[:, :R], func=CPY); out.append(t)
        return out

    def score_block(aT, bT, h):
        pc = ps()
        for k in range(4):
            MM(pc[:, :128], bT[k][:, h * 128:(h + 1) * 128],
               aT[k][:, h * 128:(h + 1) * 128], start=(k == 0), stop=(k == 3))
        e = P.tile([128, 128], BF16, tag=f"em{h}", name=f"em{h}")
        act(out=e[:], in_=pc[:, :128], func=EXP, scale=ISD)
        tt(e[:], e[:], maskbd[:], MUL)
        E = EMt
        pe = ps()
        MM(pe[:16, :128], p8[:], e[:], start=True, stop=True)
        cp(out=E[:, h * 128:(h + 1) * 128], in_=pe[:16, :128])

    def launch_ag():
        Sl = P.tile([16, 16], F32, tag="Sl", name="Sl")
        nc.vector.reduce_sum(out=Sl[:], in_=EMt.rearrange("d (b c) -> d c b", c=16),
                             axis=X)
        ib = PD.tile([16, 16], F32, tag="agin", name="agin")
        ob = PD.tile([128, 16], F32, tag="agout", name="agout")
        nc.sync.dma_start(ib[:], Sl[:])
        if collective:
            nc.gpsimd.collective_compute(
                "AllGather", BYP, ins=[ib.opt()], outs=[ob.opt()],
                replica_groups=[list(range(NC))])
        return ib, ob

    def a_rows_mm(s, prevT):
        wa, bar = w[f"wa{s}"], w[f"bar{s}"]
        a_rows = []
        for h in range(2):
            pa = ps()
            for k in range(4):
                MM(pa[:, :D1], prevT[k][:, h * 128:(h + 1) * 128], wa[:, k, :],
                   start=(k == 0), stop=False)
            MM(pa[:, :D1], ones[0:1, 0:128], bar[:], start=False, stop=True)
            t = P.tile([128, D1], BF16, tag=f"ar{h}", name=f"ar{h}")
            act(out=t[:], in_=pa[:, :D1], func=CPY); a_rows.append(t)
        return a_rows

    def head0():
        """stack0 head pipelined per batch-half: half h only needs xps[h]."""
        wa, wb = w["wa0"], w["wb0"]
        bar, bbr = w["bar0"], w["bbr0"]
        aT = [P.tile([128, R], BF16, tag=f"aT{oc}", name=f"aT{oc}")
              for oc in range(4)]
        bT = [P.tile([128, R], BF16, tag=f"bT{oc}", name=f"bT{oc}")
              for oc in range(4)]
        for h in range(2):
            for c in range(4):
                pt = PH.tile([128, 512], BF16, tag="ptb", name="ptb")
                nc.tensor.transpose(pt[:, :128],
                                    xps[h][:, c * 128:(c + 1) * 128], idn[:])
                act(out=xpT[c][:, h * 128:(h + 1) * 128], in_=pt[:, :128],
                    func=CPY)
            for wm, brow, dst in ((wa, bar, aT), (wb, bbr, bT)):
                for oc in range(4):
                    pa = ps()
                    for k in range(4):
                        MM(pa[:, :128], wm[:, k, oc * 128:(oc + 1) * 128],
                           xpT[k][:, h * 128:(h + 1) * 128],
                           start=(k == 0), stop=False)
                    MM(pa[:, :128], brow[0:1, oc * 128:(oc + 1) * 128],
                       ones[0:1, 0:128], start=False, stop=True)
                    act(out=dst[oc][:, h * 128:(h + 1) * 128], in_=pa[:, :128],
                        func=CPY)
            score_block(aT, bT, h)
        ib, ob = launch_ag()
        return aT, bT, ib, ob

    def stack_head(s, prevT, bT=None):
        wa, wb = w[f"wa{s}"], w[f"wb{s}"]
        bar, bbr = w[f"bar{s}"], w[f"bbr{s}"]
        aT = proj(wa, bar, prevT, "aT")
        if bT is None:
            bT = proj(wb, bbr, xpT, "bT")
        for h in range(2):
            score_block(aT, bT, h)
        ib, ob = launch_ag()
        return aT, ib, ob

    def stack_tail(s, prevT, a_rows, ib, ob):
        """global softmax sums -> ctx -> LN -> relu -> fc1 + residual."""
        wf, bfr = w[f"wf{s}"], w[f"bfr{s}"]
        g = P.tile([16, 8, 16], F32, tag="gsum", name="gsum")
        if collective:
            nc.sync.dma_start(g[:], ob.rearrange("(r d) c -> d r c", r=8))
        else:
            for r_ in range(8):
                nc.sync.dma_start(g[:, r_, :], ib[:])
        Sg = P.tile([16, 16], F32, tag="Sg", name="Sg")
        nc.vector.reduce_sum(out=Sg[:], in_=g.rearrange("d r c -> d c r"), axis=X)
        rS = P.tile([16, 16], BF16, tag="rS", name="rS")
        rSf = P.tile([16, 16], F32, tag="rSf", name="rSf")
        nc.vector.reciprocal(out=rSf[:], in_=Sg[:])
        cp(out=rS[:], in_=rSf[:])
        AB = P.tile([16, 256], BF16, tag="AB", name="AB")
        tt(AB.rearrange("d (b c) -> d c b", c=16),
           EMt.rearrange("d (b c) -> d c b", c=16),
           rS[:, :, None].to_broadcast((16, 16, 16)), MUL)
        bd = []
        for h in range(2):
            pb = ps()
            MM(pb[:, :128], p8t[:], AB[:, h * 128:(h + 1) * 128],
               start=True, stop=True)
            tmp = P.tile([128, 128], BF16, tag=f"bdt{h}", name=f"bdt{h}")
            act(out=tmp[:], in_=pb[:, :128], func=CPY)
            t = P.tile([128, 128], BF16, tag=f"bd{h}", name=f"bd{h}")
            tt(t[:], tmp[:], maskbd[:], MUL)
            bd.append(t)
        ctxT, sq = [], []
        for k in range(4):
            pc = ps()
            for h in range(2):
                MM(pc[:, h * 128:(h + 1) * 128],
                   a_rows[h][:, k * 128:(k + 1) * 128], bd[h][:],
                   start=True, stop=True, skip_group_check=True)
            t = P.tile([128, R], F32, tag=f"ctxT{k}", name=f"ctxT{k}")
            act(out=t[:], in_=pc[:, :R], func=CPY); ctxT.append(t)
            q = P.tile([128, R], F32, tag=f"sq{k}", name=f"sq{k}")
            act(out=q[:], in_=pc[:, :R], func=SQUARE); sq.append(q)
        ps1 = ps()
        for k in range(4):
            MM(ps1[:1, 0:R], ones128[:], ctxT[k][:], start=(k == 0),
               stop=(k == 3), skip_group_check=True)
            MM(ps1[:1, R:2 * R], ones128[:], sq[k][:], start=(k == 0),
               stop=(k == 3), skip_group_check=True)
        st = P.tile([1, 34], F32, tag="st", name="st")  # 0:16 mean 16:32 rstd 33 eps
        nc.vector.memset(st[:, 33:34], 1e-5)
        nc.vector.reduce_sum(
            out=st[:, 0:32].rearrange("o (s b) -> o s b", s=2),
            in_=ps1[:1, :2 * R].rearrange("o (s b c) -> o s b c", s=2, c=16),
            axis=X)
        nc.vector.tensor_scalar_mul(st[:, 0:32], st[:, 0:32], 1.0 / 8192.0)
        tmp = P.tile([1, 16], F32, tag="lntmp", name="lntmp")
        tt(tmp[:], st[:, 0:16], st[:, 0:16], MUL)
        tt(st[:, 16:32], st[:, 16:32], tmp[:], SUB)
        act(out=st[:, 16:32], in_=st[:, 16:32], func=SQRT, bias=st[:, 33:34])
        nc.vector.reciprocal(out=st[:, 16:32], in_=st[:, 16:32])
        pw = ps()
        MM(pw[:, :32], onesr[0:1, :], st[:, 0:32], start=True, stop=True)
        pbst = P.tile([128, 32], F32, tag="pbst", name="pbst")
        act(out=pbst[:], in_=pw[:, :32], func=CPY)
        eT = []
        for k in range(4):
            t = P.tile([128, R], F32, tag=f"eTf{k}", name=f"eTf{k}")
            v3 = t.rearrange("p (b c) -> p b c", c=16)
            tt(v3, ctxT[k].rearrange("p (b c) -> p b c", c=16),
               pbst[:, 0:16, None].to_broadcast((128, 16, 16)), SUB)
            tt(v3, v3, pbst[:, 16:32, None].to_broadcast((128, 16, 16)), MUL)
            vcb = t.rearrange("p (b c) -> p c b", c=16)
            gt(vcb, vcb, lngT[:, k, :, None].to_broadcast((128, 16, 16)), MUL)
            gt(vcb, vcb, lnbT[:, k, :, None].to_broadcast((128, 16, 16)), ADD)
            tb = P.tile([128, R], BF16, tag=f"eT{k}", name=f"eT{k}")
            act(out=tb[:], in_=t[:], func=RELU)
            eT.append(tb)
        nxT = []
        for oc in range(4):
            pn = ps()
            for k in range(4):
                MM(pn[:, :R], wf[:, k, oc * 128:(oc + 1) * 128], eT[k][:],
                   start=(k == 0), stop=False)
            MM(pn[:, :R], bfr[0:1, oc * 128:(oc + 1) * 128], ones[0:1, :R],
               start=False, stop=True)
            tmp2 = P.tile([128, R], BF16, tag=f"nxt{s}_{oc}", name=f"nxt{s}_{oc}")
            act(out=tmp2[:], in_=pn[:, :R], func=CPY)
            t = P.tile([128, R], BF16, tag=f"nxT{s}_{oc}", name=f"nxT{s}_{oc}")
            tt(t[:], tmp2[:], prevT[oc][:], ADD)
            nxT.append(t)
        return nxT

    # ---- schedule: hide each AllGather behind independent matmuls ----
    EMt = P.tile([16, 256], BF16, tag="EMt", name="EMt")
    aT0, bT0, ib0, ob0 = head0()
    a0 = a_rows_mm(0, xpT)                  # AG0 filler
    bT1 = proj(w["wb1"], w["bbr1"], xpT, "bU")  # AG0 filler: stack1 b-proj
    cls_mm(0, 48, {0: xpT})                 # more AG0 filler
    nxT0 = stack_tail(0, xpT, a0, ib0, ob0)
    aT1, ib1, ob1 = stack_head(1, nxT0, bT=bT1)
    a1 = a_rows_mm(1, nxT0)                 # AG1 filler
    cls_mm(48, 64, {0: xpT})                # more AG1 filler
    nxT1 = stack_tail(1, nxT0, a1, ib1, ob1)
    cls_mm(64, 128, {1: nxT1})
    MM(ph1[0:16, :200], ones[0:1, 0:16], b1[:], start=False, stop=True,
       tile_position=(0, 0), skip_group_check=True)
    # band-sum + transpose in one shot: h1T chunks = S_cls^T @ p8
    S_cls = P.tile([128, 200], BF16, tag="S_cls", name="S_cls")
    nc.vector.memset(S_cls[:], 0.0)
    for gb in range(4):
        act(out=S_cls[32 * gb:32 * gb + 16, :], in_=ph1[32 * gb:32 * gb + 16, :200],
            func=CPY)
    pha = ps()
    MM(pha[:, :16], S_cls[:, 0:128], p8[:], start=True, stop=True)
    h1a = P.tile([128, 16], BF16, tag="h1a", name="h1a")
    act(out=h1a[:], in_=pha[:, :16], func=RELU)
    phb = ps()
    MM(phb[:72, :16], S_cls[:, 128:200], p8[:], start=True, stop=True)
    h1b = P.tile([72, 16], BF16, tag="h1b", name="h1b")
    act(out=h1b[:], in_=phb[:72, :16], func=RELU)
    p2 = ps()
    MM(p2[:50, :BL], w2a[:], h1a[:], start=True, stop=False)
    MM(p2[:50, :BL], w2b[:], h1b[:], start=False, stop=True)
    h2 = P.tile([50, BL], BF16, tag="h2", name="h2")
    act(out=h2[:], in_=p2[:50, :BL], func=RELU, bias=b2T[:])
    p3 = ps()
    MM(p3[:10, :BL], w3[:], h2[:], start=True, stop=True)
    h3 = P.tile([10, BL], BF16, tag="h3", name="h3")
    act(out=h3[:], in_=p3[:10, :BL], func=RELU, bias=b3T[:])
    p4 = ps()
    MM(p4[:, :BL], w4[:], h3[:], start=True, stop=True)
    oT = P.tile([128, BL], F32, tag="oT", name="oT")
    tt(oT[:], p4[:, :BL], b4T[:, 0, None].to_broadcast((128, BL)), ADD)
    nc.sync.dma_start(ap["outT"], oT[:])
    ctx.close()


def build_program(collective=True):
    nc = bacc.Bacc("TRN2", target_bir_lowering=False, debug=False,
                   num_devices=NC if collective else 1)
    ap = _declare(nc)
    with tile.TileContext(nc) as tc:
        _build(nc, tc, ap, collective=collective)
    nc.compile()
    return nc


def make_in_maps(inputs):
    bf = ml_dtypes.bfloat16
    x = np.asarray(inputs["x"], np.float32)
    aw = np.asarray(inputs["attn_w"], np.float32)
    ab = np.asarray(inputs["attn_b"], np.float32)
    I16 = np.eye(16, dtype=np.float32)

    def pack_w(m):  # [out, in] -> [128, 4, out] chunked over the in dim
        return np.ascontiguousarray(
            m.T.reshape(4, 128, D1).transpose(1, 0, 2)).astype(bf)

    w1T = np.asarray(inputs["w1"], np.float32).T  # [16384, 200]
    w1p = np.ascontiguousarray(
        w1T.reshape(128, 128, 200).transpose(1, 0, 2)).astype(bf)
    shared = {
        "lngT": np.ascontiguousarray(
            np.asarray(inputs["ln_g"], np.float32).T.reshape(4, 128, D2)
            .transpose(1, 0, 2)),
        "lnbT": np.ascontiguousarray(
            np.asarray(inputs["ln_b"], np.float32).T.reshape(4, 128, D2)
            .transpose(1, 0, 2)),
        "w1a": np.ascontiguousarray(w1p[:, 0:64, :]),
        "w1b": np.ascontiguousarray(w1p[:, 64:128, :]),
        "b1": np.asarray(inputs["b1"], np.float32).reshape(1, 200).astype(bf),
        "w2T": np.ascontiguousarray(np.asarray(inputs["w2"], np.float32).T).astype(bf),
        "b2T": np.asarray(inputs["b2"], np.float32).reshape(50, 1),
        "w3T": np.ascontiguousarray(np.asarray(inputs["w3"], np.float32).T).astype(bf),
        "b3T": np.asarray(inputs["b3"], np.float32).reshape(10, 1),
        "w4T": np.ascontiguousarray(np.asarray(inputs["w4"], np.float32).T).astype(bf),
        "b4T": np.asarray(inputs["b4"], np.float32).reshape(128, 1),
        "p8": np.tile(I16, (8, 1)).astype(bf),
        "p8t": np.tile(I16, (1, 8)).astype(bf),
        "maskbd": np.kron(np.eye(8, dtype=np.float32),
                          np.ones((16, 16), np.float32)).astype(bf),
        "idn": np.eye(128, dtype=np.float32).astype(bf),
        "ones": np.ones((1, 256), np.float32).astype(bf),
        "ones128": np.ones((128, 1), np.float32),
        "onesr": np.ones((1, 128), np.float32),
    }
    for s in range(2):
        shared[f"wa{s}"] = pack_w(aw[s, 0])
        shared[f"wb{s}"] = pack_w(aw[s, 1])
        shared[f"wf{s}"] = pack_w(aw[s, 3])
        shared[f"bar{s}"] = ab[s, 0].reshape(1, D1).astype(bf)
        shared[f"bbr{s}"] = ab[s, 1].reshape(1, D1).astype(bf)
        shared[f"bfr{s}"] = ab[s, 3].reshape(1, D1).astype(bf)
    maps = []
    for r in range(NC):
        xt_ = (x[r * BL:(r + 1) * BL].transpose(0, 2, 1, 3, 4)
               .reshape(BL, D2, D1, HW))
        xs = np.concatenate([xt_, xt_[..., :1]], axis=-1)  # pad 49->50, max-safe
        xs = np.ascontiguousarray(xs.reshape(BL, D2, D1 * HWP)).astype(bf)
        maps.append({**shared, "x_s": xs})
    return maps


def kernel(**inputs):
    nc = build_program()
    in_maps = make_in_maps(inputs)
    res = run_bass_kernel_spmd(
        nc, in_maps, core_ids=list(range(NC)),
        trace=bool(os.environ.get("KTRACE")),
        tmpdir=os.environ.get("KTRACE_DIR"))
    LAST["results"] = res
    outs = [res.results[r]["outT"] for r in range(NC)]
    return np.concatenate(outs, axis=1).T.astype(np.float32)

